# revision 17
# baseline (speedup 1.0000x reference)
"""CP-decomposed conv (1x1 -> depthwise-h -> depthwise-w -> 1x1) on 8 TRN2
NeuronCores, data-parallel over batch (4 images per core).

Per-core pipeline (software-pipelined, stage B lags stage A by 2 bands):
  stage A: u[r, h', w] = sum_{a,c} x[c, h'+a, w] * W1[(a,c), r]
           (h-depthwise folded into the channel-mixing matmul via
            host-precomputed W1 = f1 (x) f3; bf16 matmuls, fp32 PSUM accum)
  ze:      ze = u * f2[0]   (ACT PSUM->SBUF copy, bf16)
  w-taps:  z = ze[+1]*r1 + ze[+0]; z = ze[+2]*r2 + z  (DVE STT)
  stage B: out[f, h', w'] = sum_r f0[f, r] * z[r, h', w']  (bf16 matmul)
  copy:    one 4D ACT copy per band moves both f-tiles PSUM->SBUF
  store:   one DMA per 2 bands (4KB descriptors) on the gpsimd queue
Output is stored bf16 in [img, ftile, p, h'*w'] layout, upcast on host.
"""

import numpy as np

B, C, H, W = 32, 256, 128, 128
FH, FW = 3, 3
F, R = 256, 128
HP, WP = H - FH + 1, W - FW + 1  # 126, 126
NCORES = 8
BL = B // NCORES  # images per core

NB = 16  # bands per image: 15 x 8 rows + 1 x 6 rows
# (last image splits its final band 3+3 so the drain chain is short)


def _bands_for(img):
    if img < BL - 1:
        return [(8 * j, 8) for j in range(15)] + [(120, 6)]
    # last image: short drain bands so the tail chain is brief
    return ([(8 * j, 8) for j in range(14)]
            + [(112, 4), (116, 4), (120, 3), (123, 3)])


def _groups_for(img):
    """Store groups: pairs of bands mid-image, singletons at the drain."""
    b = _bands_for(img)
    if img < BL - 1:
        return [[b[2 * i], b[2 * i + 1]] for i in range(8)]
    g = [[b[2 * i], b[2 * i + 1]] for i in range(7)]
    g += [[b[14]], [b[15]], [b[16]], [b[17]]]
    return g

_NC_CACHE = {}


def _build_nc():
    import concourse.bacc as bacc
    import concourse.mybir as mybir
    import concourse.tile as tile

    dt = mybir.dt
    bf16 = dt.bfloat16
    f32 = dt.float32
    mult = mybir.AluOpType.mult
    add = mybir.AluOpType.add

    nc = bacc.Bacc("TRN2", target_bir_lowering=False, debug=False,
                   num_devices=NCORES)

    x_d = nc.dram_tensor("x", [BL, C, H, W], bf16, kind="ExternalInput").ap()
    # host-prelaid SBUF layout [c_sub, kt*r]: one contiguous descriptor/partition
    w1_d = nc.dram_tensor("w1", [128, FH * 2 * R], bf16,
                          kind="ExternalInput").ap()
    f0t_d = nc.dram_tensor("f0t", [R, F], bf16, kind="ExternalInput").ap()
    # f2s[r] = [f2[0,r], f2[1,r]/f2[0,r], f2[2,r]/f2[0,r]]
    f2s_d = nc.dram_tensor("f2s", [R, FW], f32, kind="ExternalInput").ap()
    out_d = nc.dram_tensor("out", [BL, 2, 128, HP * WP], bf16,
                           kind="ExternalOutput").ap()

    with tile.TileContext(nc, trace_sim=False) as tc:
        with tc.tile_pool(name="wp", bufs=1) as wp, \
             tc.tile_pool(name="xp", bufs=2) as xp, \
             tc.tile_pool(name="ep", bufs=3) as ep, \
             tc.tile_pool(name="zp", bufs=4) as zp, \
             tc.tile_pool(name="op", bufs=3) as op, \
             tc.tile_pool(name="ups", bufs=2, space="PSUM") as upsp, \
             tc.tile_pool(name="ops", bufs=1, space="PSUM") as opsp:

            # --- weights (resident; scalar-engine DMA queue) ---
            w1_t = wp.tile([128, FH * 2, 128], bf16)  # [c_sub, kt=a*2+ct, r]
            w1v = w1_d.rearrange("p (kt r) -> p kt r", r=128)
            # first stationary early so stage A starts ASAP
            nc.scalar.dma_start(w1_t[:, 0:1, :], w1v[:, 0:1, :])
            nc.scalar.dma_start(w1_t[:, 1:FH * 2, :], w1v[:, 1:FH * 2, :])
            f2s_t = wp.tile([128, FW], f32)
            nc.scalar.dma_start(f2s_t[:, :], f2s_d)
            f0t_t = wp.tile([128, F], bf16)
            nc.scalar.dma_start(f0t_t[:, :], f0t_d)

            x_tiles = {}

            def load_image(img):
                x_t = xp.tile([128, 2, H * W], bf16, tag="x")
                x_tiles[img] = x_t
                if img == 0:
                    rchunks = [(0, 10), (10, 16), (26, 16), (42, 16),
                               (58, 16), (74, 18), (92, 18), (110, 18)]
                else:
                    rchunks = [(0, 32), (32, 32), (64, 32), (96, 32)]
                for (row0, nrow) in rchunks:
                    for ct in range(2):
                        nc.sync.dma_start(
                            x_t[:, ct, row0 * 128:(row0 + nrow) * 128],
                            x_d[img, ct * 128:(ct + 1) * 128,
                                row0:row0 + nrow, :],
                        )

            o_tiles = {}
            pend = []

            def do_B(img, h0, bh, gmeta, z_t):
                gkey, g_h0, g_rows, last_in_g = gmeta
                if gkey not in o_tiles:
                    o_tiles[gkey] = op.tile(
                        [128, 2, 2 * 8 * WP], bf16, tag="o", name="o_t")
                o_t = o_tiles[gkey]
                boff = (h0 - g_h0) * WP
                b_ps = opsp.tile([128, 2, 2, 512], f32, tag="bps")
                chunks = [(ci, 4 * ci, min(4, bh - 4 * ci))
                          for ci in range((bh + 3) // 4)]
                for ft in range(2):
                    for (ci, r0, nr) in chunks:
                        nc.tensor.matmul(
                            b_ps[:, ft, ci, 0:nr * WP],
                            f0t_t[:, ft * 128:(ft + 1) * 128],
                            z_t[:, r0 * WP:(r0 + nr) * WP],
                            start=True, stop=True,
                        )
                # PSUM->SBUF: one 4D strided copy when chunks are uniform
                if len(chunks) == 2 and chunks[0][2] == chunks[1][2]:
                    nr = chunks[0][2]
                    dst = o_t[:, :, boff:boff + bh * WP].rearrange(
                        "p f (c n) -> p f c n", n=nr * WP)
                    nc.scalar.copy(dst, b_ps[:, :, :, 0:nr * WP])
                else:
                    for (ci, r0, nr) in chunks:
                        dst = o_t[:, :,
                                  boff + r0 * WP:boff + (r0 + nr) * WP
                                  ].rearrange("p f (c n) -> p f c n",
                                              n=nr * WP)
                        nc.scalar.copy(dst, b_ps[:, :, ci:ci + 1, 0:nr * WP])
                if last_in_g:
                    nc.gpsimd.dma_start(
                        out_d[img, :, :,
                              g_h0 * WP:(g_h0 + g_rows) * WP].rearrange(
                                  "f p n -> p f n"),
                        o_t[:, :, 0:g_rows * WP],
                    )

            SEQ = []  # (img, h0, bh, gmeta)
            for img in range(BL):
                for gi, grp in enumerate(_groups_for(img)):
                    g_h0 = grp[0][0]
                    g_rows = sum(bh for _, bh in grp)
                    for bi, (h0, bh) in enumerate(grp):
                        SEQ.append((img, h0, bh,
                                    ((img, gi), g_h0, g_rows,
                                     bi == len(grp) - 1)))

            for idx, (img, h0, bh, gmeta) in enumerate(SEQ):
                # x prefetch triggers
                if idx == 0:
                    load_image(0)
                    load_image(1)
                for m in range(2, BL):
                    if idx == m * NB - 12:
                        load_image(m)

                x_t = x_tiles[img]

                # --- stage A: u in PSUM [r, bh*128]; weight-outer ---
                u_ps = upsp.tile([128, 8 * 128], f32, tag="u")
                for a in range(FH):
                    for ct in range(2):
                        for r0 in range(0, bh, 4):
                            nr = min(4, bh - r0)
                            row = h0 + r0 + a
                            nc.tensor.matmul(
                                u_ps[:, r0 * 128:r0 * 128 + nr * 128],
                                w1_t[:, a * 2 + ct, :],
                                x_t[:, ct, row * 128:(row + nr) * 128],
                                start=(a == 0 and ct == 0),
                                stop=(a == FH - 1 and ct == 1),
                            )

                # --- ze = u * f2[0] (ACT), then w-taps (DVE) ---
                # bank-split view: PSUM reads crossing a 512-f32 bank
                # boundary within one AP row are ~1.6x slower
                ze_t = ep.tile([128, 8 * 128], bf16, tag="ze")
                nc.scalar.mul(ze_t[:, 0:bh * 128], u_ps[:, 0:bh * 128],
                              f2s_t[:, 0:1])
                z_t = zp.tile([128, 8 * WP], bf16, tag="z")
                zv = z_t[:, 0:bh * WP].rearrange("p (h w) -> p h w", w=WP)
                zev = ze_t[:, 0:bh * 128].rearrange("p (h w) -> p h w", w=128)
                nc.vector.scalar_tensor_tensor(
                    zv, zev[:, :, 1:1 + WP], f2s_t[:, 1:2],
                    zev[:, :, 0:WP], op0=mult, op1=add)
                nc.vector.scalar_tensor_tensor(
                    zv, zev[:, :, 2:2 + WP], f2s_t[:, 2:3],
                    zv, op0=mult, op1=add)

                pend.append((img, h0, bh, gmeta, z_t))
                if len(pend) > 2:
                    do_B(*pend.pop(0))

            for args in pend:
                do_B(*args)

    nc.compile()
    return nc


def _get_nc():
    if "nc" not in _NC_CACHE:
        _NC_CACHE["nc"] = _build_nc()
    return _NC_CACHE["nc"]


def _prep_in_maps(x, f0, f1, f2, f3):
    import ml_dtypes
    bf16 = ml_dtypes.bfloat16

    # W1[(a, c), r] = f1[a, r] * f3[c, r], prelaid to the SBUF layout
    # [c_sub, (a*2+ct)*128 + r] so the load is one descriptor per partition
    w1 = (np.asarray(f1, np.float32)[:, None, :]
          * np.asarray(f3, np.float32)[None, :, :]).reshape(FH * C, R)
    w1b = np.ascontiguousarray(
        w1.reshape(FH, 2, 128, R).transpose(2, 0, 1, 3).reshape(128, FH * 2 * R)
        .astype(bf16))
    f0t = np.ascontiguousarray(np.asarray(f0, np.float32).T.astype(bf16))
    f2 = np.asarray(f2, np.float64)
    s0 = f2[0].copy()
    s0[np.abs(s0) < 1e-30] = 1e-30
    f2s = np.stack([s0, f2[1] / s0, f2[2] / s0], axis=1).astype(np.float32)
    f2s = np.ascontiguousarray(f2s)
    xb = np.ascontiguousarray(np.asarray(x).astype(bf16))
    return [
        {"x": xb[i * BL:(i + 1) * BL], "w1": w1b, "f0t": f0t, "f2s": f2s}
        for i in range(NCORES)
    ]


def kernel(x, f0, f1, f2, f3):
    from concourse import bass_utils

    nc = _get_nc()
    in_maps = _prep_in_maps(x, f0, f1, f2, f3)
    res = bass_utils.run_bass_kernel_spmd(
        nc, in_maps, core_ids=list(range(NCORES)))
    # out shards are [BL, 2, 128, HP*WP]; (ft, p) merges to F contiguously.
    # bf16 -> fp32 via bit shift (exact, much faster than ml_dtypes astype).
    shards = [np.asarray(r["out"]).view(np.uint16) for r in res.results]
    raw = np.stack(shards)  # [NCORES, BL, 2, 128, HP*WP] uint16
    out = (raw.astype(np.uint32) << 16).view(np.float32)
    return np.ascontiguousarray(out.reshape(B, F, HP, WP))
